# revision 3
# baseline (speedup 1.0000x reference)
"""Trainium2 Bass kernel for nn_MultiHeadAttention_901943132503 — v2.

Same sharding as v1 (core = (batch, head-group); activations replicated,
W_q/W_k/W_v column-sharded, W_o row-sharded; host sums 4 partials + b_o).

v2 changes, driven by the cost-model timeline (PE busy 335us of 553us total):
  - all matmul operands in bf16 (same 1 cycle/row as f32r, half the DMA and
    SBUF; PSUM stays f32, softmax stats stay f32)
  - software-pipelined attention: score-matmul/exp/mask of chunk i+2 issue
    before the rs/AV matmuls of chunk i, so the PE never waits on Act/Pool
  - Q/K bias folded into the PSUM->SBUF activation copy (per-partition bias);
    V bias folded into a DVE tensor_add against a pre-broadcast bias tile;
    bias seed matmuls dropped
  - softmax reciprocal broadcast via gpsimd.partition_broadcast (Pool),
    dropping the rank-1 rb matmul from the PE
  - causal diagonal computed at 256-wide granularity (6 jobs instead of 4
    full-width chunks per (I,h)), trimming ~25k wasted PE cycles
  - weight pool double-buffered so each projection's weight DMA overlaps the
    previous projection's compute
"""

import math
import os
import sys
from collections import deque
from contextlib import ExitStack

import numpy as np

for _p in ("/opt/trn_rl_repo", "/root/.axon_site/_ro/trn_rl_repo"):
    if os.path.isdir(_p) and _p not in sys.path:
        sys.path.append(_p)

import concourse.bass as bass
import concourse.mybir as mybir
import concourse.tile as tile

B, S, D = 2, 2048, 2048
H, DH = 16, 128
NCORES = 8
HPC = H // (NCORES // B)  # 4 heads per core
C = HPC * DH              # 512 channels per core
P = 128
ND = D // P               # 16 D-chunks
NQS = S // 512            # 4 q super-tiles
NKT = S // P              # 16 k chunks of 128
XS = 512                  # s-chunk width for the projection stripes
SCALE = 1.0 / math.sqrt(DH)
F32 = mybir.dt.float32
F32R = mybir.dt.float32r
BF16 = mybir.dt.bfloat16
FP8 = mybir.dt.float8e4
WSC = 64.0  # fp8 weight pre-scale (keeps W out of subnormal range)
Exp = mybir.ActivationFunctionType.Exp
Identity = mybir.ActivationFunctionType.Identity


def jobs_for_pair(plan_I):
    """Expand a chunk plan for one q super-tile into pipelined jobs.

    Each job: (kt, q0, w, mask) with mask in {None, ("tri", off), ("mix", i)}.
    512-wide jobs (full/mix chunks) come first in plan order, then 256-wide
    diagonal halves, so the first job of each accumulation region is always
    well-defined for the PSUM start flags.
    """
    full, halves = [], []
    for kt, op in plan_I:
        if op is None:
            full.append((kt, 0, 512, None))
        elif op[0] == "mix":
            full.append((kt, 0, 512, op))
        else:
            j = op[1]
            # B half (q_local 256:512): full for j<2, tri(128*(j-2)) else
            halves.append((kt, 256, 256, None if j < 2 else ("tri", 128 * (j - 2))))
            # A half (q_local 0:256): tri(128*j) for j<2, empty otherwise
            if j < 2:
                halves.append((kt, 0, 256, ("tri", 128 * j)))
    jobs = full + halves
    # PSUM start=True zeroes the ENTIRE bank (verified on hw), so exactly one
    # start per accumulation bank: the pair's chronologically first job. All
    # later jobs accumulate per-address onto the zeroed bank. stop is
    # sim-only metadata; set it on the final job.
    covered = set()
    for kt, q0, w, mask in jobs:
        covered.update(range(q0 // 256, (q0 + w) // 256))
    assert covered == {0, 1}, "a q-half has no valid keys"
    n = len(jobs)
    return [j + ((ji == 0, ji == n - 1),) for ji, j in enumerate(jobs)]


def build_program(chunk_plan, n_mixed, split_waits=True, reps=1):
    nc = bass.Bass(
        "TRN2", target_bir_lowering=False, debug=False, num_devices=NCORES
    )
    dram = {}
    for name, shape, dt in [
        ("xqT", [D, S], FP8), ("xkT", [D, S], FP8), ("xvT", [D, S], BF16),
        ("wqT", [D, C], FP8), ("wkT", [D, C], FP8), ("wvT", [D, C], BF16),
        ("woT", [C, D], BF16),
        ("bqc", [P, HPC], F32), ("bkc", [P, HPC], F32),
        ("bvb", [P, C], F32),
        ("onesr", [1, P], F32R), ("onesc", [P, 1], BF16),
    ]:
        dram[name] = nc.dram_tensor(name, shape, dt, kind="ExternalInput").ap()
    if n_mixed:
        dram["maskmix"] = nc.dram_tensor(
            "maskmix", [n_mixed, P, 512], BF16, kind="ExternalInput"
        ).ap()
    out = nc.dram_tensor("out", [S, D], F32, kind="ExternalOutput").ap()

    with tile.TileContext(nc) as tc, ExitStack() as ctx:
        persist = ctx.enter_context(tc.tile_pool(name="persist", bufs=1))
        wpool = ctx.enter_context(tc.tile_pool(name="wt", bufs=2))
        xpool = ctx.enter_context(tc.tile_pool(name="xs", bufs=3))
        ptpool = ctx.enter_context(tc.tile_pool(name="pt", bufs=8))
        opool = ctx.enter_context(tc.tile_pool(name="ot", bufs=2))
        outpool = ctx.enter_context(tc.tile_pool(name="ob", bufs=4))
        smpool = ctx.enter_context(tc.tile_pool(name="sm", bufs=1))
        rbpool = ctx.enter_context(tc.tile_pool(name="rb", bufs=1))
        psum = ctx.enter_context(tc.tile_pool(name="psum", bufs=2, space="PSUM"))
        if n_mixed:
            mixpool = ctx.enter_context(tc.tile_pool(name="mix", bufs=2))

        ones_t = persist.tile([P, 1], BF16, tag="ones")
        nc.sync.dma_start(ones_t[:], dram["onesc"][:])
        onesr = persist.tile([1, P], F32R, tag="onesr")
        nc.sync.dma_start(onesr[:], dram["onesr"][:])

        qt = persist.tile([P, HPC * S], BF16, tag="qt")   # QhT blocks: cols h*S+s
        ktile = persist.tile([P, HPC * S], BF16, tag="kt")
        vt = persist.tile([P, NKT * C], BF16, tag="vt")   # V blocks: cols st*C+c

        bqc = persist.tile([P, HPC], F32, tag="bqc")
        nc.sync.dma_start(bqc[:], dram["bqc"][:])
        bkc = persist.tile([P, HPC], F32, tag="bkc")
        nc.sync.dma_start(bkc[:], dram["bkc"][:])
        bvb = persist.tile([P, C], F32, tag="bvb")
        nc.sync.dma_start(bvb[:], dram["bvb"][:])

        def load_wT(src_ap, dt=BF16):
            t = wpool.tile([P, ND * C], dt, tag="w")
            nc.sync.dma_start(
                t[:].rearrange("p (d c) -> p d c", d=ND),
                src_ap.rearrange("(d p) c -> p d c", p=P),
            )
            return t

        def load_stripe(xT_ap, sc, dt=BF16):
            xs = xpool.tile([P, ND * XS], dt, tag="xs", name="xs")
            nc.sync.dma_start(
                xs[:].rearrange("p (d s) -> p d s", d=ND),
                xT_ap.rearrange("(d p) s -> p d s", p=P)[
                    :, :, sc * XS:(sc + 1) * XS
                ],
            )
            return xs

        def project(xT_ap, w_sb, bias_col, transposed, out_tile,
                    first_xs=None, interleave=None, dt=BF16):
            for sc in range(S // XS):
                xs = first_xs if (sc == 0 and first_xs is not None) \
                    else load_stripe(xT_ap, sc, dt)
                if transposed:
                    # out = XhT [dh, s] per head h. fp8 weights+ifmap run in
                    # DoubleRow mode (two 128-deep contractions per pass, 2x
                    # PE throughput); weights are pre-scaled by WSC on the
                    # host, undone by the activation-copy scale. Bias per
                    # partition, folded into the same PSUM->SBUF copy.
                    wr = w_sb[:].rearrange("p (d c) -> p d c", d=ND)
                    xr = xs[:].rearrange("p (d s) -> p d s", d=ND)
                    for h in range(HPC):
                        ps = psum.tile([P, XS], F32, tag="op")
                        for d2 in range(ND // 2):
                            nc.tensor.matmul(
                                ps[:],
                                wr[:, 2 * d2:2 * d2 + 2, h * DH:(h + 1) * DH],
                                xr[:, 2 * d2:2 * d2 + 2, :],
                                start=(d2 == 0), stop=(d2 == ND // 2 - 1),
                                perf_mode=mybir.MatmulPerfMode.DoubleRow,
                            )
                        nc.scalar.activation(
                            out_tile[:, h * S + sc * XS: h * S + (sc + 1) * XS],
                            ps[:], Identity, bias=bias_col[:, h:h + 1],
                            scale=1.0 / WSC,
                        )
                else:
                    # out = V natural [s, c]; bias per free element, added on
                    # DVE against the pre-broadcast bias tile.
                    for t in range(XS // P):
                        st = sc * (XS // P) + t
                        ps = psum.tile([P, C], F32, tag="op")
                        for d in range(ND):
                            nc.tensor.matmul(
                                ps[:],
                                xs[:, d * XS + t * P: d * XS + (t + 1) * P],
                                w_sb[:, d * C:(d + 1) * C],
                                start=(d == 0), stop=(d == ND - 1),
                            )
                        nc.vector.tensor_add(
                            out_tile[:, st * C:(st + 1) * C], ps[:], bvb[:]
                        )
                if sc == 0 and interleave is not None:
                    # previous rep's final out-projection: its otI/DVE tail
                    # drains while this rep's first K-proj chains run, so the
                    # PE never sits waiting on it at the rep boundary.
                    interleave()

        LOOKAHEAD = 3
        DEFER = 4  # extra backs to issue before an out-projection's chains

        wk = load_wT(dram["wkT"], FP8)
        xk0 = None
        pending_oproj = None
        for _rep in range(reps):
            project(dram["xkT"], wk, bkc, True, ktile,
                    first_xs=xk0, interleave=pending_oproj, dt=FP8)
            pending_oproj = None
            wv = load_wT(dram["wvT"])
            project(dram["xvT"], wv, None, False, vt)
            wq = load_wT(dram["wqT"], FP8)
            project(dram["xqT"], wq, bqc, True, qt, dt=FP8)

            # W_o^T slice: [C, D] -> [128, 4*2048], block h = rows of head h.
            wo = wpool.tile([P, HPC * D], BF16, tag="wo", bufs=1)
            nc.sync.dma_start(
                wo[:].rearrange("p (t j) -> p t j", t=HPC),
                dram["woT"].rearrange("(t p) j -> p t j", p=P),
            )
            if _rep + 1 < reps:
                # prefetch next rep's K weights + first x stripe during
                # attention: wq's/last-xs pool slots free once the Q
                # projection is done, so these DMAs overlap attention compute
                # instead of stalling the next rep's first chains.
                wk = load_wT(dram["wkT"], FP8)
                xk0 = load_stripe(dram["xkT"], 0, FP8)
            else:
                xk0 = None

            # ---- pipelined attention over all (I, h) pairs ----
            pair_state = {}   # pi -> dict(rs_ps, ot_ps, otI)
            otI_tiles = {}    # I -> otI tile

            def front(item):
                pi, (kt, q0, w, mask, flags) = item
                I, h = divmod(pi, HPC)
                sc_ps = psum.tile([P, w], F32, tag="sc", bufs=3, name="sc_ps")
                nc.tensor.matmul(
                    sc_ps[:],
                    ktile[:, h * S + kt * P: h * S + (kt + 1) * P],
                    qt[:, h * S + I * 512 + q0: h * S + I * 512 + q0 + w],
                    start=True, stop=True,
                )
                pt0 = ptpool.tile([P, w], BF16, tag="pt", name="pt0")
                nc.scalar.activation(pt0[:], sc_ps[:], Exp, scale=SCALE)
                if mask is None:
                    return pt0
                pt = ptpool.tile([P, w], BF16, tag="pt", name="pt")
                if mask[0] == "tri":
                    # keep pt[x, y] iff y >= x + off (q >= k)
                    nc.gpsimd.affine_select(
                        out=pt[:], in_=pt0[:],
                        compare_op=mybir.AluOpType.is_ge,
                        fill=0.0,
                        base=-mask[1],
                        channel_multiplier=-1,
                        pattern=[[1, w]],
                    )
                else:
                    mm = mixpool.tile([P, 512], BF16, tag="mix", name="mm")
                    nc.sync.dma_start(mm[:], dram["maskmix"][mask[1]])
                    nc.vector.tensor_mul(pt[:], pt0[:], mm[:])
                return pt

            def back(item, pt):
                pi, (kt, q0, w, mask, flags) = item
                I, h = divmod(pi, HPC)
                st = pair_state.get(pi)
                if st is None:
                    if h == 0:
                        otI_tiles[I] = opool.tile(
                            [P, HPC * 512], BF16, tag="ot", name="otI"
                        )
                    rs_ps = psum.tile([1, 512], F32, tag="rs", bufs=1, name="rs_ps")
                    ot_ps = psum.tile([P, 512], F32, tag="ot", name="ot_ps")
                    st = pair_state[pi] = (rs_ps, ot_ps)
                rs_ps, ot_ps = st
                first, last = flags
                nc.tensor.matmul(
                    rs_ps[:1, q0:q0 + w], ones_t[:, :1], pt[:],
                    start=first, stop=last,
                )
                nc.tensor.matmul(
                    ot_ps[:, q0:q0 + w],
                    vt[:, kt * C + h * DH: kt * C + (h + 1) * DH],
                    pt[:],
                    start=first, stop=last,
                )

            def tail(pi):
                I, h = divmod(pi, HPC)
                rs_ps, ot_ps = pair_state.pop(pi)
                rs_sb = smpool.tile([1, 512], F32, tag="rs")
                nc.scalar.copy(rs_sb[:], rs_ps[:1, :])
                rinv = smpool.tile([1, 512], F32R, tag="rinv")
                with nc.allow_low_precision("f32r reciprocal for PE broadcast"):
                    nc.vector.reciprocal(rinv[:], rs_sb[:])
                rb_ps = psum.tile([P, 512], F32, tag="rs", bufs=1, name="rb_ps")
                nc.tensor.matmul(
                    rb_ps[:], onesr[:1, :P], rinv[:], start=True, stop=True
                )
                rb_sb = rbpool.tile([P, 512], F32, tag="rb")
                nc.scalar.copy(rb_sb[:], rb_ps[:])
                otI = otI_tiles[I]
                nc.vector.tensor_mul(
                    otI[:, h * 512:(h + 1) * 512], ot_ps[:], rb_sb[:]
                )

            def oproj(I):
                otI = otI_tiles.pop(I)
                for t in range(4):
                    st = I * 4 + t
                    for jc in range(D // 512):
                        ps = psum.tile([P, 512], F32, tag="op", name="op_ps")
                        for h in range(HPC):
                            nc.tensor.matmul(
                                ps[:],
                                otI[:, h * 512 + t * P: h * 512 + (t + 1) * P],
                                wo[:, h * D + jc * 512: h * D + (jc + 1) * 512],
                                start=(h == 0), stop=(h == HPC - 1),
                            )
                        ob = outpool.tile([P, 512], F32, tag="ob")
                        nc.vector.tensor_copy(ob[:], ps[:])
                        nc.sync.dma_start(
                            out[st * P:(st + 1) * P, jc * 512:(jc + 1) * 512],
                            ob[:],
                        )

            items = []
            for I in range(NQS):
                for h in range(HPC):
                    pi = I * HPC + h
                    for job in jobs_for_pair(chunk_plan[I]):
                        items.append((pi, job))
            last_of_pair = {pi: idx for idx, (pi, _) in enumerate(items)}

            oproj_due = []  # (trigger back-idx, I) for deferred out-projs

            def drain_one(i, it, p):
                back(it, p)
                if last_of_pair[it[0]] == i:
                    tail(it[0])
                    I, h = divmod(it[0], HPC)
                    if h == HPC - 1 and I < NQS - 1:
                        oproj_due.append((i + DEFER, I))
                while oproj_due and oproj_due[0][0] <= i:
                    oproj(oproj_due.pop(0)[1])

            backlog = deque()
            for idx, item in enumerate(items):
                pt = front(item)
                backlog.append((idx, item, pt))
                if len(backlog) > LOOKAHEAD:
                    drain_one(*backlog.popleft())
            while backlog:
                drain_one(*backlog.popleft())
            while oproj_due:
                oproj(oproj_due.pop(0)[1])
            # the final super-tile's out-projection interleaves into the next
            # rep's K projection (or flushes here on the last rep)
            if _rep + 1 < reps:
                pending_oproj = lambda I=NQS - 1: oproj(I)
            else:
                oproj(NQS - 1)

    if split_waits:
        _split_matmul_waits(nc)
    return nc


def _split_matmul_waits(nc):
    """This walrus build allows at most ONE sync wait per instruction.
    Hoist all but the last wait of any multi-wait instruction onto fresh
    NoOps inserted immediately before it in the same engine stream --
    semantically identical, since the engine executes its stream in order."""
    for blk in nc.m.functions[0].blocks:
        out, changed = [], False
        for inst in blk.instructions:
            si = inst.sync_info
            if si is not None and len(si.on_wait) > 1:
                waits = list(si.on_wait)
                for w in waits[:-1]:
                    nop = mybir.InstNoOp(
                        name=nc.get_next_instruction_name(),
                        text_hint="wait_split",
                    )
                    nop.engine = inst.engine
                    nop.sync_info = mybir.SyncInfo(on_wait=[w], on_update=[])
                    out.append(nop)
                si.on_wait = [waits[-1]]
                changed = True
            out.append(inst)
        if changed:
            blk.instructions = out


def plan_from_mask(mask):
    """Classify the transposed mask in [128 k, 512 q] blocks."""
    maskT = np.ascontiguousarray(np.asarray(mask).T != 0)
    yy, xx = np.meshgrid(np.arange(512), np.arange(P))
    chunk_plan, mixed = [], []
    for I in range(NQS):
        plan_I = []
        for kt in range(NKT):
            blk = maskT[kt * P:(kt + 1) * P, I * 512:(I + 1) * 512]
            if not blk.any():
                continue
            if blk.all():
                plan_I.append((kt, None))
                continue
            j = kt - 4 * I
            if 0 <= j < 4 and np.array_equal(blk, yy >= xx + P * j):
                plan_I.append((kt, ("tri", j)))
            else:
                mixed.append(blk.astype(np.float32))
                plan_I.append((kt, ("mix", len(mixed) - 1)))
        chunk_plan.append(plan_I)
    return chunk_plan, mixed


def shard_inputs(q, k, v, W_q, b_q, W_k, b_k, W_v, b_v, W_o, mixed):
    import ml_dtypes

    bf = lambda a: np.ascontiguousarray(
        np.asarray(a, dtype=np.float32).astype(ml_dtypes.bfloat16)
    )
    f8 = lambda a: np.ascontiguousarray(
        np.asarray(a, dtype=np.float32).astype(ml_dtypes.float8_e4m3)
    )
    f32 = lambda a: np.ascontiguousarray(np.asarray(a, dtype=np.float32))
    maskmix = (np.stack(mixed).astype(ml_dtypes.bfloat16) if mixed else None)
    in_maps = []
    for core in range(NCORES):
        b, g = core // (NCORES // B), core % (NCORES // B)
        cs = slice(g * C, (g + 1) * C)
        m = {
            "xqT": f8(np.asarray(q)[b].T),
            "xkT": f8(np.asarray(k)[b].T),
            "xvT": bf(np.asarray(v)[b].T),
            "wqT": f8(WSC * np.asarray(W_q)[cs, :].T),
            "wkT": f8(WSC * np.asarray(W_k)[cs, :].T),
            "wvT": bf(np.asarray(W_v)[cs, :].T),
            "woT": bf(np.asarray(W_o)[:, cs].T),
            # per-partition bias columns: bqc[p, h] = b[g*C + h*128 + p]
            "bqc": f32(np.asarray(b_q)[cs].reshape(HPC, P).T),
            "bkc": f32(np.asarray(b_k)[cs].reshape(HPC, P).T),
            "bvb": f32(np.broadcast_to(np.asarray(b_v)[cs], (P, C))),
            "onesr": np.ones((1, P), np.float32),
            "onesc": np.ones((P, 1), ml_dtypes.bfloat16),
        }
        if maskmix is not None:
            m["maskmix"] = maskmix
        in_maps.append(m)
    return in_maps


_CACHE = {}
last_results = None


def kernel(q, k, v, mask, W_q, b_q, W_k, b_k, W_v, b_v, W_o, b_o):
    global last_results
    from concourse.bass_utils import run_bass_kernel_spmd

    mask_np = np.asarray(mask)
    assert mask_np.shape == (S, S)
    assert (mask_np != 0).any(axis=1).all(), "fully-masked rows unsupported"
    chunk_plan, mixed = plan_from_mask(mask_np)

    key = tuple(tuple(p) for p in chunk_plan)
    if key not in _CACHE:
        _CACHE[key] = build_program(chunk_plan, len(mixed))
    nc = _CACHE[key]

    in_maps = shard_inputs(q, k, v, W_q, b_q, W_k, b_k, W_v, b_v, W_o, mixed)
    trace = os.environ.get("KERNEL_TRACE", "0") == "1"
    res = run_bass_kernel_spmd(
        nc, in_maps, core_ids=list(range(NCORES)), trace=trace
    )
    last_results = res

    parts = [r["out"] for r in res.results]
    gpb = NCORES // B
    bo = np.asarray(b_o, dtype=np.float32)
    out = np.stack(
        [sum(parts[b * gpb + g] for g in range(gpb)) + bo for b in range(B)],
        axis=0,
    )
    return out.astype(np.float32)
